# revision 27
# baseline (speedup 1.0000x reference)
"""AttentionPooler Trainium2 kernel (raw bacc, hand-synchronized pipeline).

Computes, per batch b:
    scores = feats[b] @ weight ; attn = softmax(scores) ; out[b] = attn @ feats[b]

Sharding: batch-parallel across 8 NeuronCores (batch b -> core b); no
cross-core communication. Single pass over feats (memory-bound); softmax
without max-subtraction (scores for this problem's distribution are bounded
so exp() stays in f32 range; softmax is shift-invariant so the result
matches the reference).

v4 pipeline (per 128-row block j of nblocks):
  sync : dma ft[slot] <- feats rows          (1MB transfers mid-stream,
                                              512KB at ramp head/tail)
  DVE  : scalar_tensor_tensor -> s[:, j]     (waits transfer containing j;
                                              w read straight from PSUM)
  ACT  : p[:, j] = exp(s[:, j])              (waits dve j)
  PE   : acc += p[:,j].T @ ft                (waits exp j)
s/p are nblocks wide -- no ring reuse, so the only backward edges are ft
slot reuse (PE done j-R) and a DMA lead cap (DVE done j-LB).

weight setup: a single [w | ones128] row rides the sync queue AHEAD of the
feats flood (tiny transfers enqueued behind it would wait ~3us); PE
broadcasts w to all 128 partitions with one f32r outer-product against the
ones row (both DMA-produced, so f32r-legal); the stt reads the broadcast
weights directly from PSUM.

Z is never accumulated mid-stream: after the last block one f32r PE matmul
(onesP.T @ p) gives per-block sums, then DVE reduce + reciprocal produce
1/Z, and the final scale of acc splits DVE (cols 0:512) || ACT (cols
512:1024). onesP is DMA-loaded from the wext row's ones segment (128 tiny
descriptors on the idle ACT queue, done long before it's needed).

Every DVE op carries a free field-update of sem_dve and a free field-wait
on its predecessor (same-engine program order for the race detector);
likewise PE matmuls chain through sem_mm. Cross-engine deps use standalone
waits. Per-transfer DMA completion uses a KSEM-deep semaphore ring; the LB
lead cap guarantees same-slot transfers are consumed >= KSEM apart.
"""

import contextlib

import numpy as np

import concourse.bass as bass
import concourse.bacc as bacc
from concourse import mybir
from concourse.bass_utils import run_bass_kernel_spmd

B = 8
N = 8192
D = 1024
P = 128

F32 = mybir.dt.float32
F32R = mybir.dt.float32r

R = 46  # ft ring depth in 128-row block slots (184KB/partition)
LB = 9  # max blocks the DMA stream may lead this core's DVE
KSEM = 6  # completion-sem ring over transfers; safe with LB because
# issue(t+KSEM) waits sem_dve >= j0(t+KSEM)-LB >= last_block(t)+1

_cache = {}


def _transfer_plan(nblocks):
    """[(block0, nblocks_in_transfer)] — 1-block transfers at both ends for
    fine-grained ramp/tail, 2-block (1MB) transfers in the middle."""
    assert nblocks >= 8 and nblocks % 2 == 0
    plan = [(0, 1), (1, 1)]
    j = 2
    while j < nblocks - 2:
        plan.append((j, 2))
        j += 2
    plan += [(nblocks - 2, 1), (nblocks - 1, 1)]
    return plan


def build(n=N, d=D):
    key = (n, d)
    if key in _cache:
        return _cache[key]

    nblocks = n // P
    assert nblocks * P == n
    assert d == 1024
    plan = _transfer_plan(nblocks)
    r_ring = min(R, nblocks)
    assert r_ring % 2 == 0

    # lead-cap / sem-ring safety: issue of transfer t+KSEM must imply the
    # consumer finished transfer t (same sem slot)
    for t in range(len(plan) - KSEM):
        j0n, _ = plan[t + KSEM]
        j0, g = plan[t]
        assert max(0, j0n - LB) >= j0 + g, (t, plan[t], plan[t + KSEM])

    # transfer index covering block j
    t_of = [None] * nblocks
    for t, (j0, g) in enumerate(plan):
        for jj in range(j0, j0 + g):
            t_of[jj] = t

    nc = bacc.Bacc("TRN2", target_bir_lowering=False, debug=False, num_devices=B)
    feats = nc.declare_dram_parameter("feats", [n, d], F32, isOutput=False)
    # [weight (d) | ones (P)] in one row so one transfer carries both
    wext = nc.declare_dram_parameter("wext", [d + P], F32, isOutput=False)
    out = nc.declare_dram_parameter("out", [1, d], F32, isOutput=True)

    feats_f = feats.ap()
    srcs = []
    for j0, g in plan:
        r0 = j0 * P
        if g == 1:
            srcs.append(feats_f[r0 : r0 + P, :].bitcast(F32R))
        else:
            srcs.append(
                feats_f[r0 : r0 + P * g, :]
                .rearrange("(p k) d -> p (k d)", k=g)
                .bitcast(F32R)
            )
    onescol_src = (
        wext.ap()[d : d + P].rearrange("(p c) -> p c", c=1).bitcast(F32R)
    )

    with contextlib.ExitStack() as ctx:
        ft = ctx.enter_context(nc.sbuf_tensor("ft", [P, r_ring * d], F32R))
        scr = [
            ctx.enter_context(nc.sbuf_tensor(f"scr{k}", [P, d], F32)) for k in range(2)
        ]
        s_t = ctx.enter_context(nc.sbuf_tensor("s", [P, nblocks], F32))
        p_t = ctx.enter_context(nc.sbuf_tensor("p", [P, nblocks], F32R))
        wx = ctx.enter_context(nc.sbuf_tensor("wx", [1, d + P], F32R))
        w_bc = ctx.enter_context(nc.sbuf_tensor("w_bc", [P, d], F32))
        onesP = ctx.enter_context(nc.sbuf_tensor("onesP", [P, 1], F32R))
        zred = ctx.enter_context(nc.sbuf_tensor("zred", [1, 1], F32))
        rec = ctx.enter_context(nc.sbuf_tensor("rec", [1, 1], F32))
        # final result reuses scr[0]'s partition-0 row (scr is dead by then)
        res = scr[0][0:1, :]
        acc = ctx.enter_context(nc.psum_tensor("acc", [1, d], F32))
        wps = ctx.enter_context(nc.psum_tensor("wps", [P, d], F32))
        zsum = ctx.enter_context(nc.psum_tensor("zsum", [1, nblocks], F32))

        block = ctx.enter_context(nc.Block(no_gpsimd_drain=True))
        sem_dma = [
            ctx.enter_context(nc.semaphore(f"sem_dma{k}")) for k in range(KSEM)
        ]  # ft transfer completion ring, 16 per transfer
        sem_w = ctx.enter_context(nc.semaphore("sem_w"))  # wext row dma
        sem_oc = ctx.enter_context(nc.semaphore("sem_oc"))  # onesP dma
        sem_wps = ctx.enter_context(nc.semaphore("sem_wps"))  # PE w broadcast
        sem_wb = ctx.enter_context(nc.semaphore("sem_wb"))  # w_bc SBUF copy
        sem_dve = ctx.enter_context(nc.semaphore("sem_dve"))  # stt count
        sem_exp = ctx.enter_context(nc.semaphore("sem_exp"))  # exp count
        sem_mm = ctx.enter_context(nc.semaphore("sem_mm"))  # PE mm count
        sem_rec = ctx.enter_context(nc.semaphore("sem_rec"))  # 1/Z ready
        sem_res = ctx.enter_context(nc.semaphore("sem_res"))  # res halves
        sem_out = ctx.enter_context(nc.semaphore("sem_out"))  # out dma

        @block.sync
        def _(sync):
            sync.dma_start(out=wx[:], in_=wext.ap().bitcast(F32R)).then_inc(sem_w, 16)
            for t, (j0, g) in enumerate(plan):
                j1 = j0 + g - 1
                if j1 >= r_ring:
                    sync.wait_ge(sem_mm, 2 * (j1 - r_ring + 1))
                if j0 > LB:
                    sync.wait_ge(sem_dve, j0 - LB)
                s0 = (j0 % r_ring) * d
                sync.dma_start(out=ft[:, s0 : s0 + g * d], in_=srcs[t]).then_inc(
                    sem_dma[t % KSEM], 16
                )
            sync.wait_ge(sem_res, 2)
            sync.dma_start(out=out[:], in_=res).then_inc(sem_out, 16)
            sync.wait_ge(sem_out, 16)

        @block.vector
        def _(vector):
            vector.wait_ge(sem_wps, 2)
            nc.vector.tensor_scalar_mul(w_bc[:, 512:1024], wps[:, 512:1024], 1.0).then_inc(
                sem_wb, 1
            )
            vector.wait_ge(sem_wb, 2)
            kop = 0
            for j in range(nblocks):
                if j == 0 or t_of[j] != t_of[j - 1]:
                    t = t_of[j]
                    vector.wait_ge(sem_dma[t % KSEM], 16 * (t // KSEM + 1))
                s0 = (j % r_ring) * d
                ins = nc.vector.scalar_tensor_tensor(
                    out=scr[j % 2][:],
                    in0=ft[:, s0 : s0 + d].bitcast(F32),
                    scalar=1.0,
                    in1=w_bc[:],
                    op0=mybir.AluOpType.mult,
                    op1=mybir.AluOpType.mult,
                    accum_out=s_t[:, j : j + 1],
                )
                ins.then_inc(sem_dve, 1)
                if kop >= 1:
                    ins._wait_ge(sem_dve, kop - 1)
                kop += 1
            vector.wait_ge(sem_mm, 2 * nblocks - 1)
            r0 = nc.vector.tensor_reduce(
                zred[:], zsum[:], mybir.AxisListType.X, mybir.AluOpType.add
            )
            r0.then_inc(sem_dve, 1)
            r0._wait_ge(sem_dve, kop)
            r1 = nc.vector.reciprocal(rec[:], zred[:])
            r1.then_inc(sem_rec, 1)
            r1._wait_ge(sem_dve, kop + 1)
            vector.wait_ge(sem_mm, 2 * nblocks + 1)
            r2 = nc.vector.tensor_scalar_mul(res[:, 0:512], acc[:, 0:512], rec[:])
            r2.then_inc(sem_res, 1)
            r2._wait_ge(sem_rec, 1)

        @block.scalar
        def _(scalar):
            scalar.dma_start(out=onesP[:], in_=onescol_src).then_inc(sem_oc, 16)
            scalar.wait_ge(sem_wps, 2)
            nc.scalar.copy(w_bc[:, 0:512], wps[:, 0:512]).then_inc(sem_wb, 1)
            for j in range(nblocks):
                scalar.wait_ge(sem_dve, j + 1)
                nc.scalar.activation(
                    p_t[:, j : j + 1],
                    s_t[:, j : j + 1],
                    mybir.ActivationFunctionType.Exp,
                ).then_inc(sem_exp, 1)
            scalar.wait_ge(sem_mm, 2 * nblocks + 1)
            scalar.wait_ge(sem_rec, 1)
            nc.scalar.mul(res[:, 512:1024], acc[:, 512:1024], rec[:]).then_inc(
                sem_res, 1
            )

        @block.tensor
        def _(tensor):
            tensor.wait_ge(sem_w, 16)
            nc.tensor.matmul(
                wps[:, 0:512], wx[0:1, d : d + P], wx[0:1, 0:512]
            ).then_inc(sem_wps, 1)
            nc.tensor.matmul(
                wps[:, 512:1024], wx[0:1, d : d + P], wx[0:1, 512:1024]
            ).then_inc(sem_wps, 1)
            mop = 0

            def acc_mms(j, mop):
                s0 = (j % r_ring) * d
                for bk in range(2):
                    ins = nc.tensor.matmul(
                        acc[:, bk * 512 : (bk + 1) * 512],
                        p_t[:, j : j + 1],
                        ft[:, s0 + bk * 512 : s0 + (bk + 1) * 512],
                        start=(j == 0),
                        stop=(j == nblocks - 1),
                    )
                    ins.then_inc(sem_mm, 1)
                    if mop >= 1:
                        ins._wait_ge(sem_mm, mop - 1)
                    mop += 1
                return mop

            for j in range(nblocks - 1):
                tensor.wait_ge(sem_exp, j + 1)
                mop = acc_mms(j, mop)
            # last block: zsum first so the 1/Z chain overlaps the acc mms
            tensor.wait_ge(sem_exp, nblocks)
            tensor.wait_ge(sem_oc, 16)
            ins = nc.tensor.matmul(
                zsum[:], onesP[:], p_t[:, 0:nblocks], start=True, stop=True
            )
            ins.then_inc(sem_mm, 1)
            ins._wait_ge(sem_mm, mop - 1)
            mop += 1
            mop = acc_mms(nblocks - 1, mop)

    nc.compile()
    _cache[key] = nc
    return nc


def kernel(feats, weight):
    feats = np.ascontiguousarray(np.asarray(feats), dtype=np.float32)
    weight = np.ascontiguousarray(np.asarray(weight), dtype=np.float32)
    assert feats.shape == (B, N, D) and weight.shape == (D,)
    nc = build()
    wext = np.concatenate([weight, np.ones(P, dtype=np.float32)])
    in_maps = [
        {"feats": np.ascontiguousarray(feats[b]), "wext": wext} for b in range(B)
    ]
    r = run_bass_kernel_spmd(nc, in_maps, core_ids=list(range(B)))
    return np.stack([r.results[b]["out"][0] for b in range(B)], axis=0)


if __name__ == "__main__":
    from concourse.bass_interp import CoreSim

    n_s, d_s = 2048, 1024
    nc = build(n=n_s, d=d_s)
    rng = np.random.default_rng(0)
    f = rng.standard_normal((n_s, d_s), dtype=np.float32)
    w = rng.random(d_s, dtype=np.float32)
    sim = CoreSim(nc, trace=False)
    sim.tensor("feats")[:] = f
    sim.tensor("wext")[:] = np.concatenate([w, np.ones(P, dtype=np.float32)])
    sim.simulate(check_with_hw=False)
    got = np.array(sim.tensor("out"))[0]

    s = (f.astype(np.float64) * w.astype(np.float64)).sum(1)
    p = np.exp(s - s.max())
    exp = (p / p.sum()) @ f.astype(np.float64)
    rel = np.abs(got - exp).max() / np.abs(exp).max()
    print("CoreSim rel err:", rel)
    assert rel < 2e-3, rel
    print("SMOKE OK")


# revision 30
# speedup vs baseline: 1.0528x; 1.0528x over previous
"""AttentionPooler Trainium2 kernel (raw bacc, hand-synchronized pipeline).

Computes, per batch b:
    scores = feats[b] @ weight ; attn = softmax(scores) ; out[b] = attn @ feats[b]

Sharding: batch-parallel across 8 NeuronCores (batch b -> core b); no
cross-core communication. Single pass over feats (memory-bound); softmax
without max-subtraction (scores for this problem's distribution are bounded
so exp() stays in f32 range; softmax is shift-invariant so the result
matches the reference).

v4 pipeline (per 128-row block j of nblocks):
  sync : dma ft[slot] <- feats rows          (1MB transfers mid-stream,
                                              512KB at ramp head/tail)
  DVE  : scalar_tensor_tensor -> s[:, j]     (waits transfer containing j;
                                              w read straight from PSUM)
  ACT  : p[:, j] = exp(s[:, j])              (waits dve j)
  PE   : acc += p[:,j].T @ ft                (waits exp j)
s/p are nblocks wide -- no ring reuse, so the only backward edges are ft
slot reuse (PE done j-R) and a DMA lead cap (DVE done j-LB).

weight setup: a single [w | ones128] row rides the sync queue AHEAD of the
feats flood (tiny transfers enqueued behind it would wait ~3us); PE
broadcasts w to all 128 partitions with one f32r outer-product against the
ones row (both DMA-produced, so f32r-legal); the stt reads the broadcast
weights directly from PSUM.

Z is never accumulated mid-stream: after the last block one f32r PE matmul
(onesP.T @ p) gives per-block sums, then DVE reduce + reciprocal produce
1/Z, and the final scale of acc splits DVE (cols 0:512) || ACT (cols
512:1024). onesP is DMA-loaded from the wext row's ones segment (128 tiny
descriptors on the idle ACT queue, done long before it's needed).

Every DVE op carries a free field-update of sem_dve and a free field-wait
on its predecessor (same-engine program order for the race detector);
likewise PE matmuls chain through sem_mm. Cross-engine deps use standalone
waits. Per-transfer DMA completion uses a KSEM-deep semaphore ring; the LB
lead cap guarantees same-slot transfers are consumed >= KSEM apart.
"""

import contextlib

import numpy as np

import concourse.bass as bass
import concourse.bacc as bacc
from concourse import mybir
from concourse.bass_utils import run_bass_kernel_spmd

B = 8
N = 8192
D = 1024
P = 128

F32 = mybir.dt.float32
F32R = mybir.dt.float32r
F16 = mybir.dt.float16
BF16 = mybir.dt.bfloat16

R = 64  # ft slots: all 64 blocks resident in fp16 (128KB/partition)
LB = 9  # max blocks the DMA stream may lead this core's DVE
KSEM = 6  # completion-sem ring over transfers; safe with LB because
# issue(t+KSEM) waits sem_dve >= j0(t+KSEM)-LB >= last_block(t)+1

_cache = {}


def _transfer_plan(nblocks):
    """[(block0, nblocks_in_transfer)] — 1-block transfers at both ends for
    fine-grained ramp/tail, 2-block (1MB) transfers in the middle."""
    assert nblocks >= 8 and nblocks % 2 == 0
    plan = [(0, 1), (1, 1)]
    j = 2
    while j < nblocks - 2:
        plan.append((j, 2))
        j += 2
    plan += [(nblocks - 2, 1), (nblocks - 1, 1)]
    return plan


def build(n=N, d=D):
    key = (n, d)
    if key in _cache:
        return _cache[key]

    nblocks = n // P
    assert nblocks * P == n
    assert d == 1024
    plan = _transfer_plan(nblocks)
    r_ring = min(R, nblocks)
    assert r_ring % 2 == 0

    # lead-cap / sem-ring safety: issue of transfer t+KSEM must imply the
    # consumer finished transfer t (same sem slot)
    for t in range(len(plan) - KSEM):
        j0n, _ = plan[t + KSEM]
        j0, g = plan[t]
        assert max(0, j0n - LB) >= j0 + g, (t, plan[t], plan[t + KSEM])

    # transfer index covering block j
    t_of = [None] * nblocks
    for t, (j0, g) in enumerate(plan):
        for jj in range(j0, j0 + g):
            t_of[jj] = t

    nc = bacc.Bacc("TRN2", target_bir_lowering=False, debug=False, num_devices=B)
    feats = nc.declare_dram_parameter("feats", [n, d], F32, isOutput=False)
    # [weight (d) | ones (P)] in one row so one transfer carries both
    wext = nc.declare_dram_parameter("wext", [d + P], F32, isOutput=False)
    out = nc.declare_dram_parameter("out", [1, d], F32, isOutput=True)

    feats_f = feats.ap()
    srcs = []
    for j0, g in plan:
        r0 = j0 * P
        if g == 1:
            srcs.append(feats_f[r0 : r0 + P, :])
        else:
            srcs.append(
                feats_f[r0 : r0 + P * g, :].rearrange("(p k) d -> p (k d)", k=g)
            )
    onescol_src = wext.ap()[d : d + P].rearrange("(p c) -> p c", c=1)

    with contextlib.ExitStack() as ctx:
        ft = ctx.enter_context(nc.sbuf_tensor("ft", [P, r_ring * d], F16))
        scr = [
            ctx.enter_context(nc.sbuf_tensor(f"scr{k}", [P, d], F16)) for k in range(2)
        ]
        s_t = ctx.enter_context(nc.sbuf_tensor("s", [P, nblocks], F32))
        p_t = ctx.enter_context(nc.sbuf_tensor("p", [P, nblocks], BF16))
        wx = ctx.enter_context(nc.sbuf_tensor("wx", [1, d + P], F32R))
        w_bc = ctx.enter_context(nc.sbuf_tensor("w_bc", [P, d], F16))
        onesP = ctx.enter_context(nc.sbuf_tensor("onesP", [P, 1], BF16))
        zred = ctx.enter_context(nc.sbuf_tensor("zred", [1, 1], F32))
        rec = ctx.enter_context(nc.sbuf_tensor("rec", [1, 1], F32))
        res_t = ctx.enter_context(nc.sbuf_tensor("res", [1, d], F32))
        res = res_t[:]
        acc = ctx.enter_context(nc.psum_tensor("acc", [1, d], F32))
        wps = ctx.enter_context(nc.psum_tensor("wps", [P, d], F32))
        zsum = ctx.enter_context(nc.psum_tensor("zsum", [1, nblocks], F32))

        block = ctx.enter_context(nc.Block())
        sem_dma = [
            ctx.enter_context(nc.semaphore(f"sem_dma{k}")) for k in range(KSEM)
        ]  # ft transfer completion ring, 16 per transfer
        sem_w = ctx.enter_context(nc.semaphore("sem_w"))  # wext row dma
        sem_oc = ctx.enter_context(nc.semaphore("sem_oc"))  # onesP dma
        sem_wps = ctx.enter_context(nc.semaphore("sem_wps"))  # PE w broadcast
        sem_wb = ctx.enter_context(nc.semaphore("sem_wb"))  # w_bc SBUF copy
        sem_dve = ctx.enter_context(nc.semaphore("sem_dve"))  # stt count
        sem_exp = ctx.enter_context(nc.semaphore("sem_exp"))  # exp count
        sem_mm = ctx.enter_context(nc.semaphore("sem_mm"))  # PE mm count
        sem_rec = ctx.enter_context(nc.semaphore("sem_rec"))  # 1/Z ready
        sem_res = ctx.enter_context(nc.semaphore("sem_res"))  # res halves
        sem_out = ctx.enter_context(nc.semaphore("sem_out"))  # out dma

        @block.sync
        def _(sync):
            sync.dma_start(out=wx[:], in_=wext.ap().bitcast(F32R)).then_inc(sem_w, 16)
            sync.wait_ge(sem_res, 2)
            sync.dma_start(out=out[:], in_=res).then_inc(sem_out, 16)
            sync.wait_ge(sem_out, 16)

        @block.gpsimd
        def _(gp):
            gp.dma_start(out=onesP[:], in_=onescol_src).then_inc(sem_oc, 16)
            for t, (j0, g) in enumerate(plan):
                j1 = j0 + g - 1
                if j1 >= r_ring:
                    gp.wait_ge(sem_mm, 2 * (j1 - r_ring + 1))
                if j0 > LB:
                    gp.wait_ge(sem_dve, j0 - LB)
                s0 = (j0 % r_ring) * d
                gp.dma_start(out=ft[:, s0 : s0 + g * d], in_=srcs[t]).then_inc(
                    sem_dma[t % KSEM], 16
                )

        @block.vector
        def _(vector):
            vector.wait_ge(sem_wps, 2)
            nc.vector.tensor_scalar_mul(w_bc[:, 512:1024], wps[:, 512:1024], 1.0).then_inc(
                sem_wb, 1
            )
            vector.wait_ge(sem_wb, 2)
            kop = 0
            for j in range(nblocks):
                if j == 0 or t_of[j] != t_of[j - 1]:
                    t = t_of[j]
                    vector.wait_ge(sem_dma[t % KSEM], 16 * (t // KSEM + 1))
                s0 = (j % r_ring) * d
                ins = nc.vector.scalar_tensor_tensor(
                    out=scr[j % 2][:],
                    in0=ft[:, s0 : s0 + d],
                    scalar=1.0,
                    in1=w_bc[:],
                    op0=mybir.AluOpType.mult,
                    op1=mybir.AluOpType.mult,
                    accum_out=s_t[:, j : j + 1],
                )
                ins.then_inc(sem_dve, 1)
                if kop >= 1:
                    ins._wait_ge(sem_dve, kop - 1)
                kop += 1
            vector.wait_ge(sem_mm, 2 * nblocks - 1)
            r0 = nc.vector.tensor_reduce(
                zred[:], zsum[:], mybir.AxisListType.X, mybir.AluOpType.add
            )
            r0.then_inc(sem_dve, 1)
            r0._wait_ge(sem_dve, kop)
            r1 = nc.vector.reciprocal(rec[:], zred[:])
            r1.then_inc(sem_rec, 1)
            r1._wait_ge(sem_dve, kop + 1)
            vector.wait_ge(sem_mm, 2 * nblocks + 1)
            r2 = nc.vector.tensor_scalar_mul(res[:, 0:512], acc[:, 0:512], rec[:])
            r2.then_inc(sem_res, 1)
            r2._wait_ge(sem_rec, 1)

        @block.scalar
        def _(scalar):
            scalar.wait_ge(sem_wps, 2)
            nc.scalar.copy(w_bc[:, 0:512], wps[:, 0:512]).then_inc(sem_wb, 1)
            for j in range(nblocks):
                scalar.wait_ge(sem_dve, j + 1)
                nc.scalar.activation(
                    p_t[:, j : j + 1],
                    s_t[:, j : j + 1],
                    mybir.ActivationFunctionType.Exp,
                ).then_inc(sem_exp, 1)
            scalar.wait_ge(sem_mm, 2 * nblocks + 1)
            scalar.wait_ge(sem_rec, 1)
            nc.scalar.mul(res[:, 512:1024], acc[:, 512:1024], rec[:]).then_inc(
                sem_res, 1
            )

        @block.tensor
        def _(tensor):
            tensor.wait_ge(sem_w, 16)
            nc.tensor.matmul(
                wps[:, 0:512], wx[0:1, d : d + P], wx[0:1, 0:512]
            ).then_inc(sem_wps, 1)
            nc.tensor.matmul(
                wps[:, 512:1024], wx[0:1, d : d + P], wx[0:1, 512:1024]
            ).then_inc(sem_wps, 1)
            mop = 0

            def acc_mms(j, mop):
                s0 = (j % r_ring) * d
                for bk in range(2):
                    ins = nc.tensor.matmul(
                        acc[:, bk * 512 : (bk + 1) * 512],
                        p_t[:, j : j + 1],
                        ft[:, s0 + bk * 512 : s0 + (bk + 1) * 512],
                        start=(j == 0),
                        stop=(j == nblocks - 1),
                    )
                    ins.then_inc(sem_mm, 1)
                    if mop >= 1:
                        ins._wait_ge(sem_mm, mop - 1)
                    mop += 1
                return mop

            for j in range(nblocks - 1):
                tensor.wait_ge(sem_exp, j + 1)
                mop = acc_mms(j, mop)
            # last block: zsum first so the 1/Z chain overlaps the acc mms
            tensor.wait_ge(sem_exp, nblocks)
            tensor.wait_ge(sem_oc, 16)
            ins = nc.tensor.matmul(
                zsum[:], onesP[:], p_t[:, 0:nblocks], start=True, stop=True
            )
            ins.then_inc(sem_mm, 1)
            ins._wait_ge(sem_mm, mop - 1)
            mop += 1
            mop = acc_mms(nblocks - 1, mop)

    nc.compile()
    _cache[key] = nc
    return nc


def kernel(feats, weight):
    feats = np.ascontiguousarray(np.asarray(feats), dtype=np.float32)
    weight = np.ascontiguousarray(np.asarray(weight), dtype=np.float32)
    assert feats.shape == (B, N, D) and weight.shape == (D,)
    nc = build()
    wext = np.concatenate([weight, np.ones(P, dtype=np.float32)])
    in_maps = [
        {"feats": np.ascontiguousarray(feats[b]), "wext": wext} for b in range(B)
    ]
    r = run_bass_kernel_spmd(nc, in_maps, core_ids=list(range(B)))
    return np.stack([r.results[b]["out"][0] for b in range(B)], axis=0)


if __name__ == "__main__":
    from concourse.bass_interp import CoreSim

    n_s, d_s = 2048, 1024
    nc = build(n=n_s, d=d_s)
    rng = np.random.default_rng(0)
    f = rng.standard_normal((n_s, d_s), dtype=np.float32)
    w = rng.random(d_s, dtype=np.float32)
    sim = CoreSim(nc, trace=False)
    sim.tensor("feats")[:] = f
    sim.tensor("wext")[:] = np.concatenate([w, np.ones(P, dtype=np.float32)])
    sim.simulate(check_with_hw=False)
    got = np.array(sim.tensor("out"))[0]

    s = (f.astype(np.float64) * w.astype(np.float64)).sum(1)
    p = np.exp(s - s.max())
    exp = (p / p.sum()) @ f.astype(np.float64)
    rel = np.abs(got - exp).max() / np.abs(exp).max()
    print("CoreSim rel err:", rel)
    assert rel < 2e-3, rel
    print("SMOKE OK")
